# revision 29
# baseline (speedup 1.0000x reference)
"""BitConv2d (ternary-quantized 3x3 conv) on 8 Trainium2 NeuronCores.

Contract: kernel(**inputs) takes FULL unsharded inputs
  x [32, 256, 56, 56] f32, weight [256, 256, 3, 3] f32, bias [256] f32,
  scale_ema scalar f32
and returns the FULL output y [32, 256, 56, 56] f32.

Strategy: data-parallel over batch (4 images / core), weights replicated.
  Pass 1 (device): per-core max(|x_shard|) -> host combine -> beta.
  Host: quantize weights (bit-exact f32 replication of the reference
        formula) then round to fp8 e4m3, fold scalars.
  Pass 2 (device): quantize x to an EXACT fp8 pair
        x_q = x_hi + x_lo  (x_hi = e4m3 RTN of x_q, x_lo = x_q - x_hi;
        both are exactly representable in e4m3), then 3x3 conv as
        2 groups x 9 taps of fp8 DoubleRow matmuls (K=256 per matmul,
        0.5 cycles/output column -> 4x fp16 MAC throughput).  Only the
        weight's e4m3 rounding error remains (~1.0e-2 max rel err).
        Spatial tiling uses flat 58-wide padded rows: each matmul
        produces 58 columns per output row, the 2 garbage columns are
        discarded by the epilogue.
"""

import numpy as np
import ml_dtypes

import concourse.bass as bass
import concourse.tile as tile
from concourse import bacc, mybir
from concourse.bass_interp import get_hw_module
from concourse.bass_utils import run_bass_kernel_spmd

_NCORES = 8
_MAGIC = 12582912.0  # 1.5 * 2**23: adding+subtracting forces round-to-nearest-even
_F32 = mybir.dt.float32
_F16 = mybir.dt.float16
_F8 = mybir.dt.float8e4

# results of the last kernel() call, for test.py introspection
last_results = {}


def _build_max_kernel(nsh, cin, h, w):
    """Per-core abs-max over the x shard -> mx [128,1] (partition partials)."""
    nc = bacc.Bacc("TRN2", target_bir_lowering=False, debug=False,
                   num_devices=_NCORES)
    x = nc.dram_tensor("x", [nsh, cin, h, w], _F32, kind="ExternalInput")
    mx = nc.dram_tensor("mx", [128, 1], _F32, kind="ExternalOutput")
    cinc = cin // 128
    # quarter-chunk granularity keeps the reduce tail short; the final
    # (n, c) tile uses sixteenths and the first 7 tiles are pre-reduced
    # mid-stream, so only a tiny reduce chain trails the last x byte
    nq = 4
    hwq = (h * w) // nq
    nlast = 16
    hwl = (h * w) // nlast
    nfirst = (nsh * cinc - 1) * nq          # 28 quarter partials
    ntiles = nfirst + nlast + 1             # + slot for the mid-reduce
    with tile.TileContext(nc, trace_sim=False) as tc:
        with tc.tile_pool(name="xs", bufs=4) as xs, \
             tc.tile_pool(name="acc", bufs=1) as accp:
            pm = accp.tile([128, ntiles], _F32)
            k = 0
            for n in range(nsh):
                for c in range(cinc):
                    xt = xs.tile([128, h * w], _F32, name="xt", tag="xt")
                    last = (n == nsh - 1 and c == cinc - 1)
                    if last:
                        # fold the first 28 partials while the final tile's
                        # chunks are still in flight (emitted first so DVE
                        # runs it mid-stream, not on the tail)
                        nc.vector.reduce_max(pm[:, ntiles - 1:ntiles],
                                             pm[:, 0:nfirst],
                                             axis=mybir.AxisListType.X)
                    cnt, sz = (nlast, hwl) if last else (nq, hwq)
                    for q in range(cnt):
                        sl = xt[:, q * sz:(q + 1) * sz]
                        nc.sync.dma_start(
                            sl, x.ap()[n, c * 128:(c + 1) * 128]
                            .rearrange("p a b -> p (a b)")
                            [:, q * sz:(q + 1) * sz])
                        nc.vector.reduce_max(pm[:, k:k + 1], sl,
                                             axis=mybir.AxisListType.X,
                                             apply_absolute_value=True)
                        k += 1
            mxt = accp.tile([128, 1], _F32)
            nc.vector.reduce_max(mxt[:], pm[:, nfirst:ntiles],
                                 axis=mybir.AxisListType.X)
            # SWDGE out: descriptors pre-generate on Pool during the stream,
            # so the tail pays only trigger+transfer+sem
            nc.gpsimd.dma_start(mx.ap(), mxt[:])
    nc.compile()
    nc.m = get_hw_module(nc.m)
    return nc


def _build_conv_kernel(nsh, cin, cout, h, w):
    """Quantize x to exact fp8 pair + 3x3 same-pad conv, fp8 DoubleRow.

    Inputs per core:
      x  [nsh, cin, h, w] f32
      wq [9, cin//128, 128, cout] f8    (tap, ci-pair, ci, co; lhsT layout)
      b  [cout//128, 128, 1] f32
      sc [128, 2] f32                   (inv_beta, beta*gamma) broadcast rows
    Output: y [nsh, cout, h, w] f32
    """
    assert h % 8 == 0 and cin == 256
    coc = cout // 128
    hp, wp = h + 2, w + 2          # 58 x 58 padded plane
    hpa = hp + 1                    # +1 slack row: flat rhs reads 2 elems past
    plane = hpa * wp
    rowg = h // 8                   # 8-row output tiles per image
    ST = nsh * rowg

    nc = bacc.Bacc("TRN2", target_bir_lowering=False, debug=False,
                   num_devices=_NCORES)
    x = nc.dram_tensor("x", [nsh, cin, h, w], _F32, kind="ExternalInput")
    wq = nc.dram_tensor("wq", [9, 2, 128, cout], _F8, kind="ExternalInput")
    b = nc.dram_tensor("b", [coc, 128, 1], _F32, kind="ExternalInput")
    sc = nc.dram_tensor("sc", [128, 2], _F32, kind="ExternalInput")
    y = nc.dram_tensor("y", [nsh, cout, h, w], _F32, kind="ExternalOutput")

    Ident = mybir.ActivationFunctionType.Identity
    DR = mybir.MatmulPerfMode.DoubleRow

    with tile.TileContext(nc, trace_sim=False) as tc:
        with tc.tile_pool(name="const", bufs=1) as const, \
             tc.tile_pool(name="xstage",
                          bufs=int(__import__("os").environ.get(
                              "BK_XBUFS", "6"))) as xstage, \
             tc.tile_pool(name="outs", bufs=16) as outs, \
             tc.tile_pool(name="psum", bufs=8, space="PSUM") as psum:

            # ---- constants -------------------------------------------------
            # preload the ACT function table (lazy-load costs 1.3us on the
            # first activation otherwise)
            scratch = const.tile([128, 1], _F32)
            nc.scalar.activation(scratch[:],
                                 nc.const_aps.tensor(0.0, (128, 1)), Ident)
            # warm the PE while the head DMAs run: back-to-back dummy
            # matmuls on zeros keep the HAM activity window busy so the
            # first real matmuls run at 2.4GHz instead of the cold 1.2GHz
            zw = const.tile([128, 128], _F16)
            nc.vector.memset(zw[:], 0.0)
            psw = psum.tile([128, 128], _F32, name="psw", tag="ps")
            for _ in range(int(__import__("os").environ.get("BK_WARM", "54"))):
                nc.tensor.matmul(psw[:], zw[:], zw[:], start=True, stop=True)
            w_sb = const.tile([128, 9, 2, coc, 128], _F8)
            sc_sb = const.tile([128, 2], _F32)
            b_sb = const.tile([128, coc], _F32)
            mg_p = const.tile([128, 1], _F32)
            nc.vector.memset(mg_p[:], _MAGIC)

            def _load_scalars():
                # tiny scalar tiles go on the ACT-driven HWDGE queue so the
                # SP queue starts with the first x chunk immediately
                nc.scalar.dma_start(sc_sb[:], sc.ap())
                nc.scalar.dma_start(b_sb[:],
                                    b.ap().rearrange("c p o -> p (c o)"))

            def _load_weights():
                # all taps via SWDGE (desc gen on Pool, in parallel with the
                # HWDGE x stream); 3-tap pieces keep each DMA_ENGINES hold
                # short so x chunks interleave
                for t0 in range(0, 9, 3):
                    nc.gpsimd.dma_start(
                        w_sb[:, t0:t0 + 3].rearrange(
                            "p t r c m -> p t r (c m)"),
                        wq.ap()[t0:t0 + 3].rearrange("t r p f -> p t r f"))

            # ---- padded quantized input (fp8 pair, zero borders) -----------
            # layout [ci(128), pair(2), n, hpa(59), wp(58)]; row 0 and rows
            # 57-58 (pad + flat-read slack) and cols 0/57 are zero
            xq_hi = const.tile([128, 2, nsh, hpa, wp], _F8)
            xq_lo = const.tile([128, 2, nsh, hpa, wp], _F8)
            for t in (xq_hi, xq_lo):
                nc.vector.memset(t[:, :, :, 0, :], 0.0)
                nc.vector.memset(t[:, :, :, hp - 1:, :], 0.0)
                nc.vector.memset(t[:, :, :, :, 0], 0.0)
                nc.vector.memset(t[:, :, :, :, wp - 1], 0.0)

            # x_q = round_half_even(x * inv_beta); |x*inv_beta| < 127 by
            # construction so no clip is needed.
            #   W1 (ACT, in-place): t = x*inv_beta + MAGIC   (f32 RTN -> int)
            #   W2 (DVE):  x_hi = (t - MAGIC) -> e4m3        (RTN to fp8 grid)
            #   W3 (DVE/GpSimd alternating): x_lo = (t - MAGIC) - x_hi -> e4m3
            # x_hi + x_lo == x_q exactly (x_lo is a small integer <= 4).
            _load_scalars()
            _load_weights()
            state = {"qi": 0}

            def emit_quant(n):
                nch = {0: 8}.get(n, 4)
                rch = h // nch
                xts = [xstage.tile([128, h, w], _F32, name="xt", tag="xt")
                       for _ in range(2)]
                for r in range(0, h, rch):
                    for c in range(2):
                        xt = xts[c]
                        nc.sync.dma_start(
                            xt[:, r:r + rch, :],
                            x.ap()[n, c * 128:(c + 1) * 128, r:r + rch, :])
                    for c in range(2):
                        xt = xts[c]
                        nc.scalar.activation(xt[:, r:r + rch, :],
                                             xt[:, r:r + rch, :], Ident,
                                             bias=mg_p[:], scale=sc_sb[:, 0:1])
                        hi_sl = xq_hi[:, c, n, 1 + r:1 + r + rch, 1:w + 1]
                        lo_sl = xq_lo[:, c, n, 1 + r:1 + r + rch, 1:w + 1]
                        # hi-extract alternates DVE/GpSimd (stt is DVE-only)
                        eng = nc.vector if state["qi"] % 2 == 0 else nc.gpsimd
                        eng.tensor_scalar(
                            hi_sl, xt[:, r:r + rch, :], -_MAGIC, None,
                            op0=mybir.AluOpType.add)
                        nc.vector.scalar_tensor_tensor(
                            lo_sl, xt[:, r:r + rch, :], -_MAGIC, hi_sl,
                            op0=mybir.AluOpType.add,
                            op1=mybir.AluOpType.subtract)
                        state["qi"] += 1

            # ---- conv: 2 groups x 9 taps of DoubleRow matmuls per tile -----
            # rhs is a flat [128, 2, 58*nr] slice of the padded plane; each
            # output row carries 2 garbage columns (56,57) discarded by the
            # epilogue.  Groups are software-pipelined one tile apart.
            hi_flat = xq_hi[:].rearrange("p r n a b -> p r n (a b)")
            lo_flat = xq_lo[:].rearrange("p r n a b -> p r n (a b)")

            def _mm_group(ps, src, n, h0, nr, co, start, stop):
                L = wp * nr
                for tap in range(9):
                    dh, dw = tap // 3, tap % 3
                    s = (h0 + dh) * wp + dw
                    nc.tensor.matmul(
                        ps[:].rearrange("p a b -> p (a b)"),
                        w_sb[:, tap, :, co, :],
                        src[:, :, n, s:s + L],
                        start=start and tap == 0,
                        stop=stop and tap == 8,
                        perf_mode=DR)

            def _epilogue(ps, st, n, h0, nr, co, tail=False):
                ot = outs.tile([128, nr, w], _F32, name="ot", tag="ot")
                # epilogue beta*gamma*acc + bias on ACT (DVE is loaded with
                # the x_lo extraction); the tail units alternate ACT/DVE and
                # both DMA queues so the final drain chains run in parallel
                if tail and tail % 2 == 0:
                    nc.vector.tensor_scalar(ot[:], ps[:, :, 0:w],
                                            sc_sb[:, 1:2],
                                            b_sb[:, co:co + 1],
                                            op0=mybir.AluOpType.mult,
                                            op1=mybir.AluOpType.add)
                else:
                    nc.scalar.activation(ot[:], ps[:, :, 0:w], Ident,
                                         bias=b_sb[:, co:co + 1],
                                         scale=sc_sb[:, 1:2])
                # y goes out on the ACT-driven HWDGE queue: the SP queue is
                # in-order and full of x transfers, which would park every
                # y write-back behind the whole x stream.  Tail units use
                # the by-then-idle SP queue for every other write-back.
                dq = nc.sync if tail and tail % 2 == 1 else nc.scalar
                dq.dma_start(
                    y.ap()[n, co * 128:(co + 1) * 128, h0:h0 + nr, :], ot[:])

            # st-major order: each freshly quantized 8-row chunk feeds both
            # co-chunks' tiles, so the PE builds backlog instead of stalling
            units = []
            nsplit = int(__import__("os").environ.get("BK_SPLIT", "3"))
            for st in range(ST):
                for co in range(coc):
                    n, h0 = st // rowg, 8 * (st % rowg)
                    # split the trailing tiles so the tail epilogue+DMA
                    # chain after the last matmuls is short
                    if st * coc + co >= ST * coc - nsplit:
                        units.append((co, st, n, h0, 4))
                        units.append((co, st, n, h0 + 4, 4))
                    else:
                        units.append((co, st, n, h0, 8))
            # software-pipeline the EMISSION over images: quantize(img k+1)
            # is emitted before conv units(img k), so each engine's in-order
            # sequencer alternates quantize-blocks and epilogue-blocks
            # instead of parking every epilogue behind the whole quantize
            # stream (ACT head-of-line blocking stalls the PE via PSUM
            # backpressure otherwise)
            emit_quant(0)
            if nsh > 1:
                emit_quant(1)
            live = {}
            for i in range(len(units) + 1):
                if i < len(units):
                    co, st, n, h0, nr = units[i]
                    if i > 0 and units[i - 1][2] != n and n + 1 < nsh:
                        emit_quant(n + 1)
                    ps = psum.tile([128, nr, wp], _F32, name="ps", tag="ps")
                    live[i] = (ps, co, st, n, h0, nr)
                    _mm_group(ps, hi_flat, n, h0, nr, co, start=True,
                              stop=False)
                j = i - 1
                if j in live:
                    ps, co, st, n, h0, nr = live.pop(j)
                    _mm_group(ps, lo_flat, n, h0, nr, co, start=False,
                              stop=True)
                    ntail = len(units) - j  # 1 = last unit
                    _epilogue(ps, st, n, h0, nr, co,
                              tail=ntail if ntail <= 6 else 0)
    nc.compile()
    nc.m = get_hw_module(nc.m)
    return nc


_cache = {}


def _get(builder, *args):
    key = (builder.__name__,) + args
    if key not in _cache:
        _cache[key] = builder(*args)
    return _cache[key]


def _run(nc, in_maps, cores):
    """run_bass_kernel_spmd with retries for transient device errors."""
    import time
    last = None
    for attempt in range(3):
        try:
            return run_bass_kernel_spmd(nc, in_maps, cores)
        except Exception as e:
            last = e
            time.sleep(2.0 * (attempt + 1))
    raise last


def _quantize_weights(weight, gamma):
    """Bit-exact f32 replication of the reference chimera-ternary transform."""
    f32 = np.float32
    ws = (weight / gamma).astype(f32)
    tern = np.clip(np.round(ws), f32(-1.0), f32(1.0)).astype(f32)
    raw = (f32(1.0 - 0.7) * ws + f32(0.7) * tern).astype(f32)
    # straight-through estimator is an fp identity only up to rounding:
    # replicate w + (raw - w) op-for-op, then clamp
    ste = (weight + (raw - weight)).astype(f32)
    return np.clip(ste, f32(-1.0), f32(1.0)).astype(f32)


def kernel(x, weight, bias, scale_ema):
    x = np.ascontiguousarray(x, dtype=np.float32)
    weight = np.ascontiguousarray(weight, dtype=np.float32)
    bias = np.ascontiguousarray(bias, dtype=np.float32)
    f32 = np.float32
    N, cin, h, w = x.shape
    cout = weight.shape[0]
    nsh = N // _NCORES
    cores = list(range(_NCORES))

    # ---- host-side tiny prep (beta-independent, done before launch 1 so
    # the gap between the two device launches is only scalar math) ---------
    gamma = np.maximum(f32(scale_ema), f32(1e-6))
    wqf = _quantize_weights(weight, gamma)
    # [cout, cin, 3, 3] -> [tap, ci_pair, ci(128), co] fp8 e4m3 (lhsT layout)
    wql = np.ascontiguousarray(
        wqf.transpose(2, 3, 1, 0).reshape(9, 2, cin // 2, cout)
    ).astype(ml_dtypes.float8_e4m3)
    b_l = np.ascontiguousarray(bias.reshape(cout // 128, 128, 1))
    ncA = _get(_build_max_kernel, nsh, cin, h, w)
    ncB = _get(_build_conv_kernel, nsh, cin, cout, h, w)

    # ---- pass 1: global abs-max -> beta ---------------------------------
    resA = _run(ncA, [{"x": x[i * nsh:(i + 1) * nsh]} for i in cores], cores)
    last_results["max"] = resA
    gmax = f32(max(f32(r["mx"].max()) for r in resA.results))
    beta = gmax / f32(127.0) + f32(1e-6)
    sc = np.tile(np.array([f32(1.0) / beta, beta * gamma], f32), (128, 1))
    sc = np.ascontiguousarray(sc)

    # ---- pass 2: quantize x + conv --------------------------------------
    in_maps = [{"x": x[i * nsh:(i + 1) * nsh], "wq": wql, "b": b_l, "sc": sc}
               for i in cores]
    resB = _run(ncB, in_maps, cores)
    last_results["conv"] = resB
    return np.concatenate([resB.results[i]["y"] for i in cores], axis=0)


# revision 30
# speedup vs baseline: 1.0018x; 1.0018x over previous
"""BitConv2d (ternary-quantized 3x3 conv) on 8 Trainium2 NeuronCores.

Contract: kernel(**inputs) takes FULL unsharded inputs
  x [32, 256, 56, 56] f32, weight [256, 256, 3, 3] f32, bias [256] f32,
  scale_ema scalar f32
and returns the FULL output y [32, 256, 56, 56] f32.

Strategy: data-parallel over batch (4 images / core), weights replicated.
  Pass 1 (device): per-core max(|x_shard|) -> host combine -> beta.
  Host: quantize weights (bit-exact f32 replication of the reference
        formula) then round to fp8 e4m3, fold scalars.
  Pass 2 (device): quantize x to an EXACT fp8 pair
        x_q = x_hi + x_lo  (x_hi = e4m3 RTN of x_q, x_lo = x_q - x_hi;
        both are exactly representable in e4m3), then 3x3 conv as
        2 groups x 9 taps of fp8 DoubleRow matmuls (K=256 per matmul,
        0.5 cycles/output column -> 4x fp16 MAC throughput).  Only the
        weight's e4m3 rounding error remains (~1.0e-2 max rel err).
        Spatial tiling uses flat 58-wide padded rows: each matmul
        produces 58 columns per output row, the 2 garbage columns are
        discarded by the epilogue.
"""

import numpy as np
import ml_dtypes

import concourse.bass as bass
import concourse.tile as tile
from concourse import bacc, mybir
from concourse.bass_interp import get_hw_module
from concourse.bass_utils import run_bass_kernel_spmd

_NCORES = 8
_MAGIC = 12582912.0  # 1.5 * 2**23: adding+subtracting forces round-to-nearest-even
_F32 = mybir.dt.float32
_F16 = mybir.dt.float16
_F8 = mybir.dt.float8e4

# results of the last kernel() call, for test.py introspection
last_results = {}


def _build_max_kernel(nsh, cin, h, w):
    """Per-core abs-max over the x shard -> mx [128,1] (partition partials)."""
    nc = bacc.Bacc("TRN2", target_bir_lowering=False, debug=False,
                   num_devices=_NCORES)
    x = nc.dram_tensor("x", [nsh, cin, h, w], _F32, kind="ExternalInput")
    mx = nc.dram_tensor("mx", [128, 1], _F32, kind="ExternalOutput")
    cinc = cin // 128
    # quarter-chunk granularity keeps the reduce tail short; the final
    # (n, c) tile uses sixteenths and the first 7 tiles are pre-reduced
    # mid-stream, so only a tiny reduce chain trails the last x byte
    nq = 4
    hwq = (h * w) // nq
    nlast = 16
    hwl = (h * w) // nlast
    nfirst = (nsh * cinc - 1) * nq          # 28 quarter partials
    ntiles = nfirst + nlast + 1             # + slot for the mid-reduce
    with tile.TileContext(nc, trace_sim=False) as tc:
        with tc.tile_pool(name="xs", bufs=4) as xs, \
             tc.tile_pool(name="acc", bufs=1) as accp:
            pm = accp.tile([128, ntiles], _F32)
            k = 0
            for n in range(nsh):
                for c in range(cinc):
                    xt = xs.tile([128, h * w], _F32, name="xt", tag="xt")
                    last = (n == nsh - 1 and c == cinc - 1)
                    if last:
                        # fold the first 28 partials while the final tile's
                        # chunks are still in flight (emitted first so DVE
                        # runs it mid-stream, not on the tail)
                        nc.vector.reduce_max(pm[:, ntiles - 1:ntiles],
                                             pm[:, 0:nfirst],
                                             axis=mybir.AxisListType.X)
                    cnt, sz = (nlast, hwl) if last else (nq, hwq)
                    for q in range(cnt):
                        sl = xt[:, q * sz:(q + 1) * sz]
                        nc.sync.dma_start(
                            sl, x.ap()[n, c * 128:(c + 1) * 128]
                            .rearrange("p a b -> p (a b)")
                            [:, q * sz:(q + 1) * sz])
                        nc.vector.reduce_max(pm[:, k:k + 1], sl,
                                             axis=mybir.AxisListType.X,
                                             apply_absolute_value=True)
                        k += 1
            mxt = accp.tile([128, 1], _F32)
            nc.vector.reduce_max(mxt[:], pm[:, nfirst:ntiles],
                                 axis=mybir.AxisListType.X)
            nc.sync.dma_start(mx.ap(), mxt[:])
    nc.compile()
    nc.m = get_hw_module(nc.m)
    return nc


def _build_conv_kernel(nsh, cin, cout, h, w):
    """Quantize x to exact fp8 pair + 3x3 same-pad conv, fp8 DoubleRow.

    Inputs per core:
      x  [nsh, cin, h, w] f32
      wq [9, cin//128, 128, cout] f8    (tap, ci-pair, ci, co; lhsT layout)
      b  [cout//128, 128, 1] f32
      sc [128, 2] f32                   (inv_beta, beta*gamma) broadcast rows
    Output: y [nsh, cout, h, w] f32
    """
    assert h % 8 == 0 and cin == 256
    coc = cout // 128
    hp, wp = h + 2, w + 2          # 58 x 58 padded plane
    hpa = hp + 1                    # +1 slack row: flat rhs reads 2 elems past
    plane = hpa * wp
    rowg = h // 8                   # 8-row output tiles per image
    ST = nsh * rowg

    nc = bacc.Bacc("TRN2", target_bir_lowering=False, debug=False,
                   num_devices=_NCORES)
    x = nc.dram_tensor("x", [nsh, cin, h, w], _F32, kind="ExternalInput")
    wq = nc.dram_tensor("wq", [9, 2, 128, cout], _F8, kind="ExternalInput")
    b = nc.dram_tensor("b", [coc, 128, 1], _F32, kind="ExternalInput")
    sc = nc.dram_tensor("sc", [128, 2], _F32, kind="ExternalInput")
    y = nc.dram_tensor("y", [nsh, cout, h, w], _F32, kind="ExternalOutput")

    Ident = mybir.ActivationFunctionType.Identity
    DR = mybir.MatmulPerfMode.DoubleRow

    with tile.TileContext(nc, trace_sim=False) as tc:
        with tc.tile_pool(name="const", bufs=1) as const, \
             tc.tile_pool(name="xstage",
                          bufs=int(__import__("os").environ.get(
                              "BK_XBUFS", "6"))) as xstage, \
             tc.tile_pool(name="outs", bufs=16) as outs, \
             tc.tile_pool(name="psum", bufs=8, space="PSUM") as psum:

            # ---- constants -------------------------------------------------
            # preload the ACT function table (lazy-load costs 1.3us on the
            # first activation otherwise)
            scratch = const.tile([128, 1], _F32)
            nc.scalar.activation(scratch[:],
                                 nc.const_aps.tensor(0.0, (128, 1)), Ident)
            # warm the PE while the head DMAs run: back-to-back dummy
            # matmuls on zeros keep the HAM activity window busy so the
            # first real matmuls run at 2.4GHz instead of the cold 1.2GHz
            zw = const.tile([128, 128], _F16)
            nc.vector.memset(zw[:], 0.0)
            psw = psum.tile([128, 128], _F32, name="psw", tag="ps")
            for _ in range(int(__import__("os").environ.get("BK_WARM", "54"))):
                nc.tensor.matmul(psw[:], zw[:], zw[:], start=True, stop=True)
            w_sb = const.tile([128, 9, 2, coc, 128], _F8)
            sc_sb = const.tile([128, 2], _F32)
            b_sb = const.tile([128, coc], _F32)
            mg_p = const.tile([128, 1], _F32)
            nc.vector.memset(mg_p[:], _MAGIC)

            def _load_scalars():
                # tiny scalar tiles go on the ACT-driven HWDGE queue so the
                # SP queue starts with the first x chunk immediately
                nc.scalar.dma_start(sc_sb[:], sc.ap())
                nc.scalar.dma_start(b_sb[:],
                                    b.ap().rearrange("c p o -> p (c o)"))

            def _load_weights():
                # all taps via SWDGE (desc gen on Pool, in parallel with the
                # HWDGE x stream); 3-tap pieces keep each DMA_ENGINES hold
                # short so x chunks interleave
                for t0 in range(0, 9, 3):
                    nc.gpsimd.dma_start(
                        w_sb[:, t0:t0 + 3].rearrange(
                            "p t r c m -> p t r (c m)"),
                        wq.ap()[t0:t0 + 3].rearrange("t r p f -> p t r f"))

            # ---- padded quantized input (fp8 pair, zero borders) -----------
            # layout [ci(128), pair(2), n, hpa(59), wp(58)]; row 0 and rows
            # 57-58 (pad + flat-read slack) and cols 0/57 are zero
            xq_hi = const.tile([128, 2, nsh, hpa, wp], _F8)
            xq_lo = const.tile([128, 2, nsh, hpa, wp], _F8)
            for t in (xq_hi, xq_lo):
                nc.vector.memset(t[:, :, :, 0, :], 0.0)
                nc.vector.memset(t[:, :, :, hp - 1:, :], 0.0)
                nc.vector.memset(t[:, :, :, :, 0], 0.0)
                nc.vector.memset(t[:, :, :, :, wp - 1], 0.0)

            # x_q = round_half_even(x * inv_beta); |x*inv_beta| < 127 by
            # construction so no clip is needed.
            #   W1 (ACT, in-place): t = x*inv_beta + MAGIC   (f32 RTN -> int)
            #   W2 (DVE):  x_hi = (t - MAGIC) -> e4m3        (RTN to fp8 grid)
            #   W3 (DVE/GpSimd alternating): x_lo = (t - MAGIC) - x_hi -> e4m3
            # x_hi + x_lo == x_q exactly (x_lo is a small integer <= 4).
            _load_scalars()
            _load_weights()
            state = {"qi": 0}

            def emit_quant(n):
                nch = {0: 8}.get(n, 4)
                rch = h // nch
                xts = [xstage.tile([128, h, w], _F32, name="xt", tag="xt")
                       for _ in range(2)]
                for r in range(0, h, rch):
                    for c in range(2):
                        xt = xts[c]
                        nc.sync.dma_start(
                            xt[:, r:r + rch, :],
                            x.ap()[n, c * 128:(c + 1) * 128, r:r + rch, :])
                    for c in range(2):
                        xt = xts[c]
                        nc.scalar.activation(xt[:, r:r + rch, :],
                                             xt[:, r:r + rch, :], Ident,
                                             bias=mg_p[:], scale=sc_sb[:, 0:1])
                        hi_sl = xq_hi[:, c, n, 1 + r:1 + r + rch, 1:w + 1]
                        lo_sl = xq_lo[:, c, n, 1 + r:1 + r + rch, 1:w + 1]
                        # hi-extract alternates DVE/GpSimd (stt is DVE-only)
                        eng = nc.vector if state["qi"] % 2 == 0 else nc.gpsimd
                        eng.tensor_scalar(
                            hi_sl, xt[:, r:r + rch, :], -_MAGIC, None,
                            op0=mybir.AluOpType.add)
                        nc.vector.scalar_tensor_tensor(
                            lo_sl, xt[:, r:r + rch, :], -_MAGIC, hi_sl,
                            op0=mybir.AluOpType.add,
                            op1=mybir.AluOpType.subtract)
                        state["qi"] += 1

            # ---- conv: 2 groups x 9 taps of DoubleRow matmuls per tile -----
            # rhs is a flat [128, 2, 58*nr] slice of the padded plane; each
            # output row carries 2 garbage columns (56,57) discarded by the
            # epilogue.  Groups are software-pipelined one tile apart.
            hi_flat = xq_hi[:].rearrange("p r n a b -> p r n (a b)")
            lo_flat = xq_lo[:].rearrange("p r n a b -> p r n (a b)")

            def _mm_group(ps, src, n, h0, nr, co, start, stop):
                L = wp * nr
                for tap in range(9):
                    dh, dw = tap // 3, tap % 3
                    s = (h0 + dh) * wp + dw
                    nc.tensor.matmul(
                        ps[:].rearrange("p a b -> p (a b)"),
                        w_sb[:, tap, :, co, :],
                        src[:, :, n, s:s + L],
                        start=start and tap == 0,
                        stop=stop and tap == 8,
                        perf_mode=DR)

            def _epilogue(ps, st, n, h0, nr, co, tail=False):
                ot = outs.tile([128, nr, w], _F32, name="ot", tag="ot")
                # epilogue beta*gamma*acc + bias on ACT (DVE is loaded with
                # the x_lo extraction); the tail units alternate ACT/DVE and
                # both DMA queues so the final drain chains run in parallel
                if tail and tail % 2 == 0:
                    nc.vector.tensor_scalar(ot[:], ps[:, :, 0:w],
                                            sc_sb[:, 1:2],
                                            b_sb[:, co:co + 1],
                                            op0=mybir.AluOpType.mult,
                                            op1=mybir.AluOpType.add)
                else:
                    nc.scalar.activation(ot[:], ps[:, :, 0:w], Ident,
                                         bias=b_sb[:, co:co + 1],
                                         scale=sc_sb[:, 1:2])
                # y goes out on the ACT-driven HWDGE queue: the SP queue is
                # in-order and full of x transfers, which would park every
                # y write-back behind the whole x stream.  Tail units use
                # the by-then-idle SP queue for every other write-back.
                dq = nc.sync if tail and tail % 2 == 1 else nc.scalar
                dq.dma_start(
                    y.ap()[n, co * 128:(co + 1) * 128, h0:h0 + nr, :], ot[:])

            # st-major order: each freshly quantized 8-row chunk feeds both
            # co-chunks' tiles, so the PE builds backlog instead of stalling
            units = []
            nsplit = int(__import__("os").environ.get("BK_SPLIT", "3"))
            for st in range(ST):
                for co in range(coc):
                    n, h0 = st // rowg, 8 * (st % rowg)
                    # split the trailing tiles so the tail epilogue+DMA
                    # chain after the last matmuls is short
                    if st * coc + co >= ST * coc - nsplit:
                        units.append((co, st, n, h0, 4))
                        units.append((co, st, n, h0 + 4, 4))
                    else:
                        units.append((co, st, n, h0, 8))
            # software-pipeline the EMISSION over images: quantize(img k+1)
            # is emitted before conv units(img k), so each engine's in-order
            # sequencer alternates quantize-blocks and epilogue-blocks
            # instead of parking every epilogue behind the whole quantize
            # stream (ACT head-of-line blocking stalls the PE via PSUM
            # backpressure otherwise)
            emit_quant(0)
            if nsh > 1:
                emit_quant(1)
            live = {}
            for i in range(len(units) + 1):
                if i < len(units):
                    co, st, n, h0, nr = units[i]
                    if i > 0 and units[i - 1][2] != n and n + 1 < nsh:
                        emit_quant(n + 1)
                    ps = psum.tile([128, nr, wp], _F32, name="ps", tag="ps")
                    live[i] = (ps, co, st, n, h0, nr)
                    _mm_group(ps, hi_flat, n, h0, nr, co, start=True,
                              stop=False)
                j = i - 1
                if j in live:
                    ps, co, st, n, h0, nr = live.pop(j)
                    _mm_group(ps, lo_flat, n, h0, nr, co, start=False,
                              stop=True)
                    ntail = len(units) - j  # 1 = last unit
                    _epilogue(ps, st, n, h0, nr, co,
                              tail=ntail if ntail <= 6 else 0)
    nc.compile()
    nc.m = get_hw_module(nc.m)
    return nc


_cache = {}


def _get(builder, *args):
    key = (builder.__name__,) + args
    if key not in _cache:
        _cache[key] = builder(*args)
    return _cache[key]


def _run(nc, in_maps, cores):
    """run_bass_kernel_spmd with retries for transient device errors."""
    import time
    last = None
    for attempt in range(3):
        try:
            return run_bass_kernel_spmd(nc, in_maps, cores)
        except Exception as e:
            last = e
            time.sleep(2.0 * (attempt + 1))
    raise last


def _quantize_weights(weight, gamma):
    """Bit-exact f32 replication of the reference chimera-ternary transform."""
    f32 = np.float32
    ws = (weight / gamma).astype(f32)
    tern = np.clip(np.round(ws), f32(-1.0), f32(1.0)).astype(f32)
    raw = (f32(1.0 - 0.7) * ws + f32(0.7) * tern).astype(f32)
    # straight-through estimator is an fp identity only up to rounding:
    # replicate w + (raw - w) op-for-op, then clamp
    ste = (weight + (raw - weight)).astype(f32)
    return np.clip(ste, f32(-1.0), f32(1.0)).astype(f32)


def kernel(x, weight, bias, scale_ema):
    x = np.ascontiguousarray(x, dtype=np.float32)
    weight = np.ascontiguousarray(weight, dtype=np.float32)
    bias = np.ascontiguousarray(bias, dtype=np.float32)
    f32 = np.float32
    N, cin, h, w = x.shape
    cout = weight.shape[0]
    nsh = N // _NCORES
    cores = list(range(_NCORES))

    # ---- host-side tiny prep (beta-independent, done before launch 1 so
    # the gap between the two device launches is only scalar math) ---------
    gamma = np.maximum(f32(scale_ema), f32(1e-6))
    wqf = _quantize_weights(weight, gamma)
    # [cout, cin, 3, 3] -> [tap, ci_pair, ci(128), co] fp8 e4m3 (lhsT layout)
    wql = np.ascontiguousarray(
        wqf.transpose(2, 3, 1, 0).reshape(9, 2, cin // 2, cout)
    ).astype(ml_dtypes.float8_e4m3)
    b_l = np.ascontiguousarray(bias.reshape(cout // 128, 128, 1))
    ncA = _get(_build_max_kernel, nsh, cin, h, w)
    ncB = _get(_build_conv_kernel, nsh, cin, cout, h, w)

    # ---- pass 1: global abs-max -> beta ---------------------------------
    resA = _run(ncA, [{"x": x[i * nsh:(i + 1) * nsh]} for i in cores], cores)
    last_results["max"] = resA
    gmax = f32(max(f32(r["mx"].max()) for r in resA.results))
    beta = gmax / f32(127.0) + f32(1e-6)
    sc = np.tile(np.array([f32(1.0) / beta, beta * gamma], f32), (128, 1))
    sc = np.ascontiguousarray(sc)

    # ---- pass 2: quantize x + conv --------------------------------------
    in_maps = [{"x": x[i * nsh:(i + 1) * nsh], "wq": wql, "b": b_l, "sc": sc}
               for i in cores]
    resB = _run(ncB, in_maps, cores)
    last_results["conv"] = resB
    return np.concatenate([resB.results[i]["y"] for i in cores], axis=0)


# revision 32
# speedup vs baseline: 1.0194x; 1.0175x over previous
"""BitConv2d (ternary-quantized 3x3 conv) on 8 Trainium2 NeuronCores.

Contract: kernel(**inputs) takes FULL unsharded inputs
  x [32, 256, 56, 56] f32, weight [256, 256, 3, 3] f32, bias [256] f32,
  scale_ema scalar f32
and returns the FULL output y [32, 256, 56, 56] f32.

Strategy: data-parallel over batch (4 images / core), weights replicated.
  Pass 1 (device): per-core max(|x_shard|) -> host combine -> beta.
  Host: quantize weights (bit-exact f32 replication of the reference
        formula) then round to fp8 e4m3, fold scalars.
  Pass 2 (device): quantize x to an EXACT fp8 pair
        x_q = x_hi + x_lo  (x_hi = e4m3 RTN of x_q, x_lo = x_q - x_hi;
        both are exactly representable in e4m3), then 3x3 conv as
        2 groups x 9 taps of fp8 DoubleRow matmuls (K=256 per matmul,
        0.5 cycles/output column -> 4x fp16 MAC throughput).  Only the
        weight's e4m3 rounding error remains (~1.0e-2 max rel err).
        Spatial tiling uses flat 58-wide padded rows: each matmul
        produces 58 columns per output row, the 2 garbage columns are
        discarded by the epilogue.
"""

import numpy as np
import ml_dtypes

import concourse.bass as bass
import concourse.tile as tile
from concourse import bacc, mybir
from concourse.bass_interp import get_hw_module
from concourse.bass_utils import run_bass_kernel_spmd

_NCORES = 8
_MAGIC = 12582912.0  # 1.5 * 2**23: adding+subtracting forces round-to-nearest-even
_F32 = mybir.dt.float32
_F16 = mybir.dt.float16
_F8 = mybir.dt.float8e4

# results of the last kernel() call, for test.py introspection
last_results = {}


def _build_max_kernel(nsh, cin, h, w):
    """Per-core abs-max over the x shard -> mx [128,1] (partition partials)."""
    nc = bacc.Bacc("TRN2", target_bir_lowering=False, debug=False,
                   num_devices=_NCORES)
    x = nc.dram_tensor("x", [nsh, cin, h, w], _F32, kind="ExternalInput")
    mx = nc.dram_tensor("mx", [128, 1], _F32, kind="ExternalOutput")
    cinc = cin // 128
    # quarter-chunk granularity keeps the reduce tail short; the final
    # (n, c) tile uses sixteenths and the first 7 tiles are pre-reduced
    # mid-stream, so only a tiny reduce chain trails the last x byte
    nq = 4
    hwq = (h * w) // nq
    # tapered last tile: big chunks are transfer-bound, the trailing small
    # ones shrink the post-stream reduce tail without going HWDGE-gen-bound
    last_sizes = [hwq, hwq, hwq, hwq // 2, hwq // 4, hwq // 4]
    assert sum(last_sizes) == h * w
    nfirst = (nsh * cinc - 1) * nq          # 28 quarter partials
    ntiles = nfirst + len(last_sizes) + 1   # + slot for the mid-reduce
    with tile.TileContext(nc, trace_sim=False) as tc:
        with tc.tile_pool(name="xs", bufs=4) as xs, \
             tc.tile_pool(name="acc", bufs=1) as accp:
            pm = accp.tile([128, ntiles], _F32)
            k = 0
            for n in range(nsh):
                for c in range(cinc):
                    xt = xs.tile([128, h * w], _F32, name="xt", tag="xt")
                    last = (n == nsh - 1 and c == cinc - 1)
                    if last:
                        # fold the first 28 partials while the final tile's
                        # chunks are still in flight (emitted first so DVE
                        # runs it mid-stream, not on the tail)
                        nc.vector.reduce_max(pm[:, ntiles - 1:ntiles],
                                             pm[:, 0:nfirst],
                                             axis=mybir.AxisListType.X)
                    sizes = last_sizes if last else [hwq] * nq
                    off = 0
                    for sz in sizes:
                        sl = xt[:, off:off + sz]
                        nc.sync.dma_start(
                            sl, x.ap()[n, c * 128:(c + 1) * 128]
                            .rearrange("p a b -> p (a b)")
                            [:, off:off + sz])
                        nc.vector.reduce_max(pm[:, k:k + 1], sl,
                                             axis=mybir.AxisListType.X,
                                             apply_absolute_value=True)
                        k += 1
                        off += sz
            mxt = accp.tile([128, 1], _F32)
            nc.vector.reduce_max(mxt[:], pm[:, nfirst:ntiles],
                                 axis=mybir.AxisListType.X)
            nc.sync.dma_start(mx.ap(), mxt[:])
    nc.compile()
    nc.m = get_hw_module(nc.m)
    return nc


def _build_conv_kernel(nsh, cin, cout, h, w):
    """Quantize x to exact fp8 pair + 3x3 same-pad conv, fp8 DoubleRow.

    Inputs per core:
      x  [nsh, cin, h, w] f32
      wq [9, cin//128, 128, cout] f8    (tap, ci-pair, ci, co; lhsT layout)
      b  [cout//128, 128, 1] f32
      sc [128, 2] f32                   (inv_beta, beta*gamma) broadcast rows
    Output: y [nsh, cout, h, w] f32
    """
    assert h % 8 == 0 and cin == 256
    coc = cout // 128
    hp, wp = h + 2, w + 2          # 58 x 58 padded plane
    hpa = hp + 1                    # +1 slack row: flat rhs reads 2 elems past
    plane = hpa * wp
    rowg = h // 8                   # 8-row output tiles per image
    ST = nsh * rowg

    nc = bacc.Bacc("TRN2", target_bir_lowering=False, debug=False,
                   num_devices=_NCORES)
    x = nc.dram_tensor("x", [nsh, cin, h, w], _F32, kind="ExternalInput")
    wq = nc.dram_tensor("wq", [9, 2, 128, cout], _F8, kind="ExternalInput")
    b = nc.dram_tensor("b", [coc, 128, 1], _F32, kind="ExternalInput")
    sc = nc.dram_tensor("sc", [128, 2], _F32, kind="ExternalInput")
    y = nc.dram_tensor("y", [nsh, cout, h, w], _F32, kind="ExternalOutput")

    Ident = mybir.ActivationFunctionType.Identity
    DR = mybir.MatmulPerfMode.DoubleRow

    with tile.TileContext(nc, trace_sim=False) as tc:
        with tc.tile_pool(name="const", bufs=1) as const, \
             tc.tile_pool(name="xstage",
                          bufs=int(__import__("os").environ.get(
                              "BK_XBUFS", "6"))) as xstage, \
             tc.tile_pool(name="outs", bufs=16) as outs, \
             tc.tile_pool(name="psum", bufs=8, space="PSUM") as psum:

            # ---- constants -------------------------------------------------
            # preload the ACT function table (lazy-load costs 1.3us on the
            # first activation otherwise)
            scratch = const.tile([128, 1], _F32)
            nc.scalar.activation(scratch[:],
                                 nc.const_aps.tensor(0.0, (128, 1)), Ident)
            # warm the PE while the head DMAs run: back-to-back dummy
            # matmuls on zeros keep the HAM activity window busy so the
            # first real matmuls run at 2.4GHz instead of the cold 1.2GHz
            zw = const.tile([128, 128], _F16)
            nc.vector.memset(zw[:], 0.0)
            psw = psum.tile([128, 128], _F32, name="psw", tag="ps")
            for _ in range(int(__import__("os").environ.get("BK_WARM", "54"))):
                nc.tensor.matmul(psw[:], zw[:], zw[:], start=True, stop=True)
            w_sb = const.tile([128, 9, 2, coc, 128], _F8)
            sc_sb = const.tile([128, 2], _F32)
            b_sb = const.tile([128, coc], _F32)
            mg_p = const.tile([128, 1], _F32)
            nc.vector.memset(mg_p[:], _MAGIC)

            def _load_scalars():
                # tiny scalar tiles go on the ACT-driven HWDGE queue so the
                # SP queue starts with the first x chunk immediately
                nc.scalar.dma_start(sc_sb[:], sc.ap())
                nc.scalar.dma_start(b_sb[:],
                                    b.ap().rearrange("c p o -> p (c o)"))

            def _load_weights():
                # all taps via SWDGE (desc gen on Pool, in parallel with the
                # HWDGE x stream); 3-tap pieces keep each DMA_ENGINES hold
                # short so x chunks interleave
                for t0 in range(0, 9, 3):
                    nc.gpsimd.dma_start(
                        w_sb[:, t0:t0 + 3].rearrange(
                            "p t r c m -> p t r (c m)"),
                        wq.ap()[t0:t0 + 3].rearrange("t r p f -> p t r f"))

            # ---- padded quantized input (fp8 pair, zero borders) -----------
            # layout [ci(128), pair(2), n, hpa(59), wp(58)]; row 0 and rows
            # 57-58 (pad + flat-read slack) and cols 0/57 are zero
            xq_hi = const.tile([128, 2, nsh, hpa, wp], _F8)
            xq_lo = const.tile([128, 2, nsh, hpa, wp], _F8)
            for t in (xq_hi, xq_lo):
                nc.vector.memset(t[:, :, :, 0, :], 0.0)
                nc.vector.memset(t[:, :, :, hp - 1:, :], 0.0)
                nc.vector.memset(t[:, :, :, :, 0], 0.0)
                nc.vector.memset(t[:, :, :, :, wp - 1], 0.0)

            # x_q = round_half_even(x * inv_beta); |x*inv_beta| < 127 by
            # construction so no clip is needed.
            #   W1 (ACT, in-place): t = x*inv_beta + MAGIC   (f32 RTN -> int)
            #   W2 (DVE):  x_hi = (t - MAGIC) -> e4m3        (RTN to fp8 grid)
            #   W3 (DVE/GpSimd alternating): x_lo = (t - MAGIC) - x_hi -> e4m3
            # x_hi + x_lo == x_q exactly (x_lo is a small integer <= 4).
            _load_scalars()
            _load_weights()
            state = {"qi": 0}

            def emit_quant(n):
                nch = {0: 8}.get(n, 4)
                rch = h // nch
                xts = [xstage.tile([128, h, w], _F32, name="xt", tag="xt")
                       for _ in range(2)]
                for r in range(0, h, rch):
                    for c in range(2):
                        xt = xts[c]
                        nc.sync.dma_start(
                            xt[:, r:r + rch, :],
                            x.ap()[n, c * 128:(c + 1) * 128, r:r + rch, :])
                    for c in range(2):
                        xt = xts[c]
                        nc.scalar.activation(xt[:, r:r + rch, :],
                                             xt[:, r:r + rch, :], Ident,
                                             bias=mg_p[:], scale=sc_sb[:, 0:1])
                        hi_sl = xq_hi[:, c, n, 1 + r:1 + r + rch, 1:w + 1]
                        lo_sl = xq_lo[:, c, n, 1 + r:1 + r + rch, 1:w + 1]
                        # hi-extract alternates DVE/GpSimd (stt is DVE-only)
                        eng = nc.vector if state["qi"] % 2 == 0 else nc.gpsimd
                        eng.tensor_scalar(
                            hi_sl, xt[:, r:r + rch, :], -_MAGIC, None,
                            op0=mybir.AluOpType.add)
                        nc.vector.scalar_tensor_tensor(
                            lo_sl, xt[:, r:r + rch, :], -_MAGIC, hi_sl,
                            op0=mybir.AluOpType.add,
                            op1=mybir.AluOpType.subtract)
                        state["qi"] += 1

            # ---- conv: 2 groups x 9 taps of DoubleRow matmuls per tile -----
            # rhs is a flat [128, 2, 58*nr] slice of the padded plane; each
            # output row carries 2 garbage columns (56,57) discarded by the
            # epilogue.  Groups are software-pipelined one tile apart.
            hi_flat = xq_hi[:].rearrange("p r n a b -> p r n (a b)")
            lo_flat = xq_lo[:].rearrange("p r n a b -> p r n (a b)")

            def _mm_group(ps, src, n, h0, nr, co, start, stop):
                L = wp * nr
                for tap in range(9):
                    dh, dw = tap // 3, tap % 3
                    s = (h0 + dh) * wp + dw
                    nc.tensor.matmul(
                        ps[:].rearrange("p a b -> p (a b)"),
                        w_sb[:, tap, :, co, :],
                        src[:, :, n, s:s + L],
                        start=start and tap == 0,
                        stop=stop and tap == 8,
                        perf_mode=DR)

            def _epilogue(ps, st, n, h0, nr, co, tail=False):
                ot = outs.tile([128, nr, w], _F32, name="ot", tag="ot")
                # epilogue beta*gamma*acc + bias on ACT (DVE is loaded with
                # the x_lo extraction); the tail units alternate ACT/DVE and
                # both DMA queues so the final drain chains run in parallel
                if tail and tail % 2 == 0:
                    nc.vector.tensor_scalar(ot[:], ps[:, :, 0:w],
                                            sc_sb[:, 1:2],
                                            b_sb[:, co:co + 1],
                                            op0=mybir.AluOpType.mult,
                                            op1=mybir.AluOpType.add)
                else:
                    nc.scalar.activation(ot[:], ps[:, :, 0:w], Ident,
                                         bias=b_sb[:, co:co + 1],
                                         scale=sc_sb[:, 1:2])
                # y goes out on the ACT-driven HWDGE queue: the SP queue is
                # in-order and full of x transfers, which would park every
                # y write-back behind the whole x stream.  Tail units use
                # the by-then-idle SP queue for every other write-back.
                dq = nc.sync if tail and tail % 2 == 1 else nc.scalar
                dq.dma_start(
                    y.ap()[n, co * 128:(co + 1) * 128, h0:h0 + nr, :], ot[:])

            # st-major order: each freshly quantized 8-row chunk feeds both
            # co-chunks' tiles, so the PE builds backlog instead of stalling
            units = []
            nsplit = int(__import__("os").environ.get("BK_SPLIT", "3"))
            for st in range(ST):
                for co in range(coc):
                    n, h0 = st // rowg, 8 * (st % rowg)
                    # split the trailing tiles so the tail epilogue+DMA
                    # chain after the last matmuls is short
                    if st * coc + co >= ST * coc - nsplit:
                        units.append((co, st, n, h0, 4))
                        units.append((co, st, n, h0 + 4, 4))
                    else:
                        units.append((co, st, n, h0, 8))
            # software-pipeline the EMISSION over images: quantize(img k+1)
            # is emitted before conv units(img k), so each engine's in-order
            # sequencer alternates quantize-blocks and epilogue-blocks
            # instead of parking every epilogue behind the whole quantize
            # stream (ACT head-of-line blocking stalls the PE via PSUM
            # backpressure otherwise)
            emit_quant(0)
            if nsh > 1:
                emit_quant(1)
            live = {}
            for i in range(len(units) + 1):
                if i < len(units):
                    co, st, n, h0, nr = units[i]
                    if i > 0 and units[i - 1][2] != n and n + 1 < nsh:
                        emit_quant(n + 1)
                    ps = psum.tile([128, nr, wp], _F32, name="ps", tag="ps")
                    live[i] = (ps, co, st, n, h0, nr)
                    _mm_group(ps, hi_flat, n, h0, nr, co, start=True,
                              stop=False)
                j = i - 1
                if j in live:
                    ps, co, st, n, h0, nr = live.pop(j)
                    _mm_group(ps, lo_flat, n, h0, nr, co, start=False,
                              stop=True)
                    ntail = len(units) - j  # 1 = last unit
                    _epilogue(ps, st, n, h0, nr, co,
                              tail=ntail if ntail <= 6 else 0)
    nc.compile()
    nc.m = get_hw_module(nc.m)
    return nc


_cache = {}


def _get(builder, *args):
    key = (builder.__name__,) + args
    if key not in _cache:
        _cache[key] = builder(*args)
    return _cache[key]


def _run(nc, in_maps, cores):
    """run_bass_kernel_spmd with retries for transient device errors."""
    import time
    last = None
    for attempt in range(3):
        try:
            return run_bass_kernel_spmd(nc, in_maps, cores)
        except Exception as e:
            last = e
            time.sleep(2.0 * (attempt + 1))
    raise last


def _quantize_weights(weight, gamma):
    """Bit-exact f32 replication of the reference chimera-ternary transform."""
    f32 = np.float32
    ws = (weight / gamma).astype(f32)
    tern = np.clip(np.round(ws), f32(-1.0), f32(1.0)).astype(f32)
    raw = (f32(1.0 - 0.7) * ws + f32(0.7) * tern).astype(f32)
    # straight-through estimator is an fp identity only up to rounding:
    # replicate w + (raw - w) op-for-op, then clamp
    ste = (weight + (raw - weight)).astype(f32)
    return np.clip(ste, f32(-1.0), f32(1.0)).astype(f32)


def kernel(x, weight, bias, scale_ema):
    x = np.ascontiguousarray(x, dtype=np.float32)
    weight = np.ascontiguousarray(weight, dtype=np.float32)
    bias = np.ascontiguousarray(bias, dtype=np.float32)
    f32 = np.float32
    N, cin, h, w = x.shape
    cout = weight.shape[0]
    nsh = N // _NCORES
    cores = list(range(_NCORES))

    # ---- host-side tiny prep (beta-independent, done before launch 1 so
    # the gap between the two device launches is only scalar math) ---------
    gamma = np.maximum(f32(scale_ema), f32(1e-6))
    wqf = _quantize_weights(weight, gamma)
    # [cout, cin, 3, 3] -> [tap, ci_pair, ci(128), co] fp8 e4m3 (lhsT layout)
    wql = np.ascontiguousarray(
        wqf.transpose(2, 3, 1, 0).reshape(9, 2, cin // 2, cout)
    ).astype(ml_dtypes.float8_e4m3)
    b_l = np.ascontiguousarray(bias.reshape(cout // 128, 128, 1))
    ncA = _get(_build_max_kernel, nsh, cin, h, w)
    ncB = _get(_build_conv_kernel, nsh, cin, cout, h, w)

    # ---- pass 1: global abs-max -> beta ---------------------------------
    resA = _run(ncA, [{"x": x[i * nsh:(i + 1) * nsh]} for i in cores], cores)
    last_results["max"] = resA
    gmax = f32(max(f32(r["mx"].max()) for r in resA.results))
    beta = gmax / f32(127.0) + f32(1e-6)
    sc = np.tile(np.array([f32(1.0) / beta, beta * gamma], f32), (128, 1))
    sc = np.ascontiguousarray(sc)

    # ---- pass 2: quantize x + conv --------------------------------------
    in_maps = [{"x": x[i * nsh:(i + 1) * nsh], "wq": wql, "b": b_l, "sc": sc}
               for i in cores]
    resB = _run(ncB, in_maps, cores)
    last_results["conv"] = resB
    return np.concatenate([resB.results[i]["y"] for i in cores], axis=0)


# revision 34
# speedup vs baseline: 1.0388x; 1.0190x over previous
"""BitConv2d (ternary-quantized 3x3 conv) on 8 Trainium2 NeuronCores.

Contract: kernel(**inputs) takes FULL unsharded inputs
  x [32, 256, 56, 56] f32, weight [256, 256, 3, 3] f32, bias [256] f32,
  scale_ema scalar f32
and returns the FULL output y [32, 256, 56, 56] f32.

Strategy: data-parallel over batch (4 images / core), weights replicated.
  Pass 1 (device): per-core max(|x_shard|) -> host combine -> beta.
  Host: quantize weights (bit-exact f32 replication of the reference
        formula) then round to fp8 e4m3, fold scalars.
  Pass 2 (device): quantize x to an EXACT fp8 pair
        x_q = x_hi + x_lo  (x_hi = e4m3 RTN of x_q, x_lo = x_q - x_hi;
        both are exactly representable in e4m3), then 3x3 conv as
        2 groups x 9 taps of fp8 DoubleRow matmuls (K=256 per matmul,
        0.5 cycles/output column -> 4x fp16 MAC throughput).  Only the
        weight's e4m3 rounding error remains (~1.0e-2 max rel err).
        Spatial tiling uses flat 58-wide padded rows: each matmul
        produces 58 columns per output row, the 2 garbage columns are
        discarded by the epilogue.
"""

import numpy as np
import ml_dtypes

import concourse.bass as bass
import concourse.tile as tile
from concourse import bacc, mybir
from concourse.bass_interp import get_hw_module
from concourse.bass_utils import run_bass_kernel_spmd

_NCORES = 8
_MAGIC = 12582912.0  # 1.5 * 2**23: adding+subtracting forces round-to-nearest-even
_F32 = mybir.dt.float32
_F16 = mybir.dt.float16
_F8 = mybir.dt.float8e4

# results of the last kernel() call, for test.py introspection
last_results = {}


def _build_max_kernel(nsh, cin, h, w):
    """Per-core abs-max over the x shard -> mx [128,1] (partition partials)."""
    nc = bacc.Bacc("TRN2", target_bir_lowering=False, debug=False,
                   num_devices=_NCORES)
    x = nc.dram_tensor("x", [nsh, cin, h, w], _F32, kind="ExternalInput")
    mx = nc.dram_tensor("mx", [128, 1], _F32, kind="ExternalOutput")
    cinc = cin // 128
    # quarter-chunk granularity keeps the reduce tail short; the final
    # (n, c) tile uses sixteenths and the first 7 tiles are pre-reduced
    # mid-stream, so only a tiny reduce chain trails the last x byte
    nq = 4
    hwq = (h * w) // nq
    # tapered last tile: big chunks are transfer-bound, the trailing small
    # ones shrink the post-stream reduce tail without going HWDGE-gen-bound
    last_sizes = [hwq, hwq, hwq, hwq // 2, hwq // 4, hwq // 4]
    assert sum(last_sizes) == h * w
    nfirst = (nsh * cinc - 1) * nq          # 28 quarter partials
    ntiles = nfirst + len(last_sizes) + 1   # + slot for the mid-reduce
    with tile.TileContext(nc, trace_sim=False) as tc:
        with tc.tile_pool(name="xs", bufs=4) as xs, \
             tc.tile_pool(name="acc", bufs=1) as accp:
            pm = accp.tile([128, ntiles], _F32)
            k = 0
            for n in range(nsh):
                for c in range(cinc):
                    xt = xs.tile([128, h * w], _F32, name="xt", tag="xt")
                    last = (n == nsh - 1 and c == cinc - 1)
                    if last:
                        # fold the first 28 partials while the final tile's
                        # chunks are still in flight (emitted first so DVE
                        # runs it mid-stream, not on the tail)
                        nc.vector.reduce_max(pm[:, ntiles - 1:ntiles],
                                             pm[:, 0:nfirst],
                                             axis=mybir.AxisListType.X)
                    sizes = last_sizes if last else [hwq] * nq
                    off = 0
                    for sz in sizes:
                        sl = xt[:, off:off + sz]
                        nc.sync.dma_start(
                            sl, x.ap()[n, c * 128:(c + 1) * 128]
                            .rearrange("p a b -> p (a b)")
                            [:, off:off + sz])
                        nc.vector.reduce_max(pm[:, k:k + 1], sl,
                                             axis=mybir.AxisListType.X,
                                             apply_absolute_value=True)
                        k += 1
                        off += sz
            mxt = accp.tile([128, 1], _F32)
            nc.vector.reduce_max(mxt[:], pm[:, nfirst:ntiles],
                                 axis=mybir.AxisListType.X)
            nc.sync.dma_start(mx.ap(), mxt[:])
    nc.compile()
    nc.m = get_hw_module(nc.m)
    return nc


def _build_conv_kernel(nsh, cin, cout, h, w):
    """Quantize x to exact fp8 pair + 3x3 same-pad conv, fp8 DoubleRow.

    Inputs per core:
      x  [nsh, cin, h, w] f32
      wq [9, cin//128, 128, cout] f8    (tap, ci-pair, ci, co; lhsT layout)
      b  [cout//128, 128, 1] f32
      sc [128, 2] f32                   (inv_beta, beta*gamma) broadcast rows
    Output: y [nsh, cout, h, w] f32
    """
    assert h % 8 == 0 and cin == 256
    coc = cout // 128
    hp, wp = h + 2, w + 2          # 58 x 58 padded plane
    hpa = hp + 1                    # +1 slack row: flat rhs reads 2 elems past
    plane = hpa * wp
    rowg = h // 8                   # 8-row output tiles per image
    ST = nsh * rowg

    nc = bacc.Bacc("TRN2", target_bir_lowering=False, debug=False,
                   num_devices=_NCORES)
    x = nc.dram_tensor("x", [nsh, cin, h, w], _F32, kind="ExternalInput")
    wq = nc.dram_tensor("wq", [9, 2, 128, cout], _F8, kind="ExternalInput")
    b = nc.dram_tensor("b", [coc, 128, 1], _F32, kind="ExternalInput")
    sc = nc.dram_tensor("sc", [128, 2], _F32, kind="ExternalInput")
    y = nc.dram_tensor("y", [nsh, cout, h, w], _F32, kind="ExternalOutput")

    Ident = mybir.ActivationFunctionType.Identity
    DR = mybir.MatmulPerfMode.DoubleRow

    with tile.TileContext(nc, trace_sim=False) as tc:
        with tc.tile_pool(name="const", bufs=1) as const, \
             tc.tile_pool(name="xstage",
                          bufs=int(__import__("os").environ.get(
                              "BK_XBUFS", "6"))) as xstage, \
             tc.tile_pool(name="outs", bufs=16) as outs, \
             tc.tile_pool(name="psum", bufs=8, space="PSUM") as psum:

            # ---- constants -------------------------------------------------
            # preload the ACT function table (lazy-load costs 1.3us on the
            # first activation otherwise)
            scratch = const.tile([128, 1], _F32)
            nc.scalar.activation(scratch[:],
                                 nc.const_aps.tensor(0.0, (128, 1)), Ident)
            # warm the PE while the head DMAs run: back-to-back dummy
            # matmuls on zeros keep the HAM activity window busy so the
            # first real matmuls run at 2.4GHz instead of the cold 1.2GHz
            zw = const.tile([128, 128], _F16)
            nc.vector.memset(zw[:], 0.0)
            psw = psum.tile([128, 128], _F32, name="psw", tag="ps")
            for _ in range(int(__import__("os").environ.get("BK_WARM", "54"))):
                nc.tensor.matmul(psw[:], zw[:], zw[:], start=True, stop=True)
            w_sb = const.tile([128, 9, 2, coc, 128], _F8)
            sc_sb = const.tile([128, 2], _F32)
            b_sb = const.tile([128, coc], _F32)
            mg_p = const.tile([128, 1], _F32)
            nc.vector.memset(mg_p[:], _MAGIC)

            def _load_scalars():
                # tiny scalar tiles go on the ACT-driven HWDGE queue so the
                # SP queue starts with the first x chunk immediately
                nc.scalar.dma_start(sc_sb[:], sc.ap())
                nc.scalar.dma_start(b_sb[:],
                                    b.ap().rearrange("c p o -> p (c o)"))

            def _load_weights():
                # all taps via SWDGE (desc gen on Pool, in parallel with the
                # HWDGE x stream); 3-tap pieces keep each DMA_ENGINES hold
                # short so x chunks interleave
                for t0 in range(0, 9, 3):
                    nc.gpsimd.dma_start(
                        w_sb[:, t0:t0 + 3].rearrange(
                            "p t r c m -> p t r (c m)"),
                        wq.ap()[t0:t0 + 3].rearrange("t r p f -> p t r f"))

            # ---- padded quantized input (fp8 pair, zero borders) -----------
            # layout [ci(128), pair(2), n, hpa(59), wp(58)]; row 0 and rows
            # 57-58 (pad + flat-read slack) and cols 0/57 are zero
            xq_hi = const.tile([128, 2, nsh, hpa, wp], _F8)
            xq_lo = const.tile([128, 2, nsh, hpa, wp], _F8)
            for t in (xq_hi, xq_lo):
                nc.vector.memset(t[:, :, :, 0, :], 0.0)
                nc.vector.memset(t[:, :, :, hp - 1:, :], 0.0)
                nc.vector.memset(t[:, :, :, :, 0], 0.0)
                nc.vector.memset(t[:, :, :, :, wp - 1], 0.0)

            # x_q = round_half_even(x * inv_beta); |x*inv_beta| < 127 by
            # construction so no clip is needed.
            #   W1 (ACT, in-place): t = x*inv_beta + MAGIC   (f32 RTN -> int)
            #   W2 (DVE):  x_hi = (t - MAGIC) -> e4m3        (RTN to fp8 grid)
            #   W3 (DVE/GpSimd alternating): x_lo = (t - MAGIC) - x_hi -> e4m3
            # x_hi + x_lo == x_q exactly (x_lo is a small integer <= 4).
            _load_scalars()
            _load_weights()
            state = {"qi": 0}

            def emit_quant(n):
                nch = {0: 8}.get(n, 4)
                rch = h // nch
                xts = [xstage.tile([128, h, w], _F32, name="xt", tag="xt")
                       for _ in range(2)]
                for r in range(0, h, rch):
                    for c in range(2):
                        xt = xts[c]
                        nc.sync.dma_start(
                            xt[:, r:r + rch, :],
                            x.ap()[n, c * 128:(c + 1) * 128, r:r + rch, :])
                    for c in range(2):
                        xt = xts[c]
                        # W1 (magic round) for images 2-3 runs on GpSimd:
                        # on ACT it would queue AHEAD of image-1's epilogues
                        # (in-order sequencer) while waiting on late x
                        # chunks, stalling the PE via PSUM backpressure
                        if n < 2:
                            nc.scalar.activation(xt[:, r:r + rch, :],
                                                 xt[:, r:r + rch, :], Ident,
                                                 bias=mg_p[:],
                                                 scale=sc_sb[:, 0:1])
                        else:
                            nc.gpsimd.tensor_scalar(
                                xt[:, r:r + rch, :], xt[:, r:r + rch, :],
                                sc_sb[:, 0:1], mg_p[:],
                                op0=mybir.AluOpType.mult,
                                op1=mybir.AluOpType.add)
                        hi_sl = xq_hi[:, c, n, 1 + r:1 + r + rch, 1:w + 1]
                        lo_sl = xq_lo[:, c, n, 1 + r:1 + r + rch, 1:w + 1]
                        # hi-extract alternates DVE/GpSimd (stt is DVE-only);
                        # images 2-3 keep it on DVE since GpSimd runs their W1
                        eng = (nc.vector if state["qi"] % 2 == 0 or n >= 2
                               else nc.gpsimd)
                        eng.tensor_scalar(
                            hi_sl, xt[:, r:r + rch, :], -_MAGIC, None,
                            op0=mybir.AluOpType.add)
                        nc.vector.scalar_tensor_tensor(
                            lo_sl, xt[:, r:r + rch, :], -_MAGIC, hi_sl,
                            op0=mybir.AluOpType.add,
                            op1=mybir.AluOpType.subtract)
                        state["qi"] += 1

            # ---- conv: 2 groups x 9 taps of DoubleRow matmuls per tile -----
            # rhs is a flat [128, 2, 58*nr] slice of the padded plane; each
            # output row carries 2 garbage columns (56,57) discarded by the
            # epilogue.  Groups are software-pipelined one tile apart.
            hi_flat = xq_hi[:].rearrange("p r n a b -> p r n (a b)")
            lo_flat = xq_lo[:].rearrange("p r n a b -> p r n (a b)")

            def _mm_group(ps, src, n, h0, nr, co, start, stop):
                L = wp * nr
                for tap in range(9):
                    dh, dw = tap // 3, tap % 3
                    s = (h0 + dh) * wp + dw
                    nc.tensor.matmul(
                        ps[:].rearrange("p a b -> p (a b)"),
                        w_sb[:, tap, :, co, :],
                        src[:, :, n, s:s + L],
                        start=start and tap == 0,
                        stop=stop and tap == 8,
                        perf_mode=DR)

            def _epilogue(ps, st, n, h0, nr, co, tail=False):
                ot = outs.tile([128, nr, w], _F32, name="ot", tag="ot")
                # epilogue beta*gamma*acc + bias on ACT (DVE is loaded with
                # the x_lo extraction); the tail units alternate ACT/DVE and
                # both DMA queues so the final drain chains run in parallel
                if tail and tail % 2 == 0:
                    nc.vector.tensor_scalar(ot[:], ps[:, :, 0:w],
                                            sc_sb[:, 1:2],
                                            b_sb[:, co:co + 1],
                                            op0=mybir.AluOpType.mult,
                                            op1=mybir.AluOpType.add)
                else:
                    nc.scalar.activation(ot[:], ps[:, :, 0:w], Ident,
                                         bias=b_sb[:, co:co + 1],
                                         scale=sc_sb[:, 1:2])
                # y goes out on the ACT-driven HWDGE queue: the SP queue is
                # in-order and full of x transfers, which would park every
                # y write-back behind the whole x stream.  Tail units use
                # the by-then-idle SP queue for every other write-back.
                dq = nc.sync if tail and tail % 2 == 1 else nc.scalar
                dq.dma_start(
                    y.ap()[n, co * 128:(co + 1) * 128, h0:h0 + nr, :], ot[:])

            # st-major order: each freshly quantized 8-row chunk feeds both
            # co-chunks' tiles, so the PE builds backlog instead of stalling
            units = []
            nsplit = int(__import__("os").environ.get("BK_SPLIT", "3"))
            for st in range(ST):
                for co in range(coc):
                    n, h0 = st // rowg, 8 * (st % rowg)
                    # split the trailing tiles so the tail epilogue+DMA
                    # chain after the last matmuls is short
                    if st * coc + co >= ST * coc - nsplit:
                        units.append((co, st, n, h0, 4))
                        units.append((co, st, n, h0 + 4, 4))
                    else:
                        units.append((co, st, n, h0, 8))
            # software-pipeline the EMISSION over images: quantize(img k+1)
            # is emitted before conv units(img k), so each engine's in-order
            # sequencer alternates quantize-blocks and epilogue-blocks
            # instead of parking every epilogue behind the whole quantize
            # stream (ACT head-of-line blocking stalls the PE via PSUM
            # backpressure otherwise)
            emit_quant(0)
            if nsh > 1:
                emit_quant(1)
            live = {}
            for i in range(len(units) + 1):
                if i < len(units):
                    co, st, n, h0, nr = units[i]
                    if i > 0 and units[i - 1][2] != n and n + 1 < nsh:
                        emit_quant(n + 1)
                    ps = psum.tile([128, nr, wp], _F32, name="ps", tag="ps")
                    live[i] = (ps, co, st, n, h0, nr)
                    _mm_group(ps, hi_flat, n, h0, nr, co, start=True,
                              stop=False)
                j = i - 1
                if j in live:
                    ps, co, st, n, h0, nr = live.pop(j)
                    _mm_group(ps, lo_flat, n, h0, nr, co, start=False,
                              stop=True)
                    ntail = len(units) - j  # 1 = last unit
                    _epilogue(ps, st, n, h0, nr, co,
                              tail=ntail if ntail <= 6 else 0)
    nc.compile()
    nc.m = get_hw_module(nc.m)
    return nc


_cache = {}


def _get(builder, *args):
    key = (builder.__name__,) + args
    if key not in _cache:
        _cache[key] = builder(*args)
    return _cache[key]


def _run(nc, in_maps, cores):
    """run_bass_kernel_spmd with retries for transient device errors."""
    import time
    last = None
    for attempt in range(3):
        try:
            return run_bass_kernel_spmd(nc, in_maps, cores)
        except Exception as e:
            last = e
            time.sleep(2.0 * (attempt + 1))
    raise last


def _quantize_weights(weight, gamma):
    """Bit-exact f32 replication of the reference chimera-ternary transform."""
    f32 = np.float32
    ws = (weight / gamma).astype(f32)
    tern = np.clip(np.round(ws), f32(-1.0), f32(1.0)).astype(f32)
    raw = (f32(1.0 - 0.7) * ws + f32(0.7) * tern).astype(f32)
    # straight-through estimator is an fp identity only up to rounding:
    # replicate w + (raw - w) op-for-op, then clamp
    ste = (weight + (raw - weight)).astype(f32)
    return np.clip(ste, f32(-1.0), f32(1.0)).astype(f32)


def kernel(x, weight, bias, scale_ema):
    x = np.ascontiguousarray(x, dtype=np.float32)
    weight = np.ascontiguousarray(weight, dtype=np.float32)
    bias = np.ascontiguousarray(bias, dtype=np.float32)
    f32 = np.float32
    N, cin, h, w = x.shape
    cout = weight.shape[0]
    nsh = N // _NCORES
    cores = list(range(_NCORES))

    # ---- host-side tiny prep (beta-independent, done before launch 1 so
    # the gap between the two device launches is only scalar math) ---------
    gamma = np.maximum(f32(scale_ema), f32(1e-6))
    wqf = _quantize_weights(weight, gamma)
    # [cout, cin, 3, 3] -> [tap, ci_pair, ci(128), co] fp8 e4m3 (lhsT layout)
    wql = np.ascontiguousarray(
        wqf.transpose(2, 3, 1, 0).reshape(9, 2, cin // 2, cout)
    ).astype(ml_dtypes.float8_e4m3)
    b_l = np.ascontiguousarray(bias.reshape(cout // 128, 128, 1))
    ncA = _get(_build_max_kernel, nsh, cin, h, w)
    ncB = _get(_build_conv_kernel, nsh, cin, cout, h, w)

    # ---- pass 1: global abs-max -> beta ---------------------------------
    resA = _run(ncA, [{"x": x[i * nsh:(i + 1) * nsh]} for i in cores], cores)
    last_results["max"] = resA
    gmax = f32(max(f32(r["mx"].max()) for r in resA.results))
    beta = gmax / f32(127.0) + f32(1e-6)
    sc = np.tile(np.array([f32(1.0) / beta, beta * gamma], f32), (128, 1))
    sc = np.ascontiguousarray(sc)

    # ---- pass 2: quantize x + conv --------------------------------------
    in_maps = [{"x": x[i * nsh:(i + 1) * nsh], "wq": wql, "b": b_l, "sc": sc}
               for i in cores]
    resB = _run(ncB, in_maps, cores)
    last_results["conv"] = resB
    return np.concatenate([resB.results[i]["y"] for i in cores], axis=0)
